# revision 5
# baseline (speedup 1.0000x reference)
"""ComplexPolarAttention Trainium2 kernel (8-core SPMD, row-sharded).

Math (matching the reference):
  c = mag*cos(phase); s = mag*sin(phase)
  scores = c@c.T + s@s.T + bias     (bias: sparse edge scatter, last-dup-wins)
  attn = softmax(scores, axis=1)
  out = (attn@mag, attn@phase)

Design (v3, position-hybrid bias):
  exp(S+B) = exp(S) * exp(B).  For query block 0 the bias is applied
  ADDITIVELY before the exp (GpSimd scatters the sparse edge scores
  into dense f16 tiles, DVE adds them onto the QK PSUM) — this needs
  only tiny index tables early, keeping the DMA ramp light.  For query
  blocks 1..3 the bias is applied MULTIPLICATIVELY after the exp with a
  host-precomputed dense M = exp(B) in bf16, streamed during the long
  steady-state window on the sync+gpsimd DGE queues.

  The scalar (ACT) queue carries NO DMA configs; ACT does nothing but
  stream 64 back-to-back [128,1024] exps (~1.0us each) — it is the
  pacing engine.  QK matmuls for quad q+1 are emitted before PV matmuls
  of quad q so the tensor engine never stalls on the exp.

Device per core (1024 query rows), per group g (= 4 key chunks x 256
queries = [128 dst, 1024] score tile):
  PE:   S^T group tile = xt_kc.T @ xtq   (f16, PSUM f32)
  qb0:  GpSimd local_scatter bias -> DVE add (PSUM+f16 -> f32 SBUF)
        -> ACT exp -> p
  qb1+: ACT exp straight from PSUM -> DVE p = exp(S) * M (bf16 2x)
  PE:   pv[128 q, 129] += p_chunk.T @ [mag|phase|ones]  (col 128 =
        softmax denominator); epilogue divides and DMAs out.
"""
import os
import sys

sys.path.insert(0, "/opt/trn_rl_repo")

# The NTFF profile hook module is missing from this image's antenv package;
# bass_utils imports it unconditionally when tracing. Create it if absent so
# BASS_TRACE=1 works (degrades silently if dirs are read-only).
_HOOK_SRC = '''_hook = None

def set_axon_ntff_profile_hook(hook):
    global _hook
    _hook = hook

def get_axon_ntff_profile_hook():
    return _hook
'''
for _d in ("/opt/trn_rl_repo/antenv", "/root/.axon_site/_ro/trn_rl_repo/antenv"):
    try:
        _p = os.path.join(_d, "axon_hooks.py")
        if os.path.isdir(_d) and not os.path.exists(_p):
            with open(_p, "w") as _f:
                _f.write(_HOOK_SRC)
    except OSError:
        pass

import numpy as np
import ml_dtypes

import concourse.bass as bass
import concourse.mybir as mybir
import concourse.tile as tile
from concourse import bacc
from concourse.bass_utils import run_bass_kernel_spmd

N, D, E, EDGE_DIM = 8192, 64, 262144, 4
CORES = 8
NQ = N // CORES          # 1024 query rows per core
QB_W = 256               # query block width
N_QB = NQ // QB_W        # 4 query blocks per core
KC = 128                 # key chunk (dst) width
N_KC = N // KC           # 64 key chunks
KCG = 4                  # key chunks per group
N_G = N_KC // KCG        # 16 groups per qb
GW = KCG * QB_W          # 1024 = group tile width
QUAD = 4                 # groups per QK-emission batch
N_Q4 = N_G // QUAD       # 4 quads per qb
MPW = 132                # padded [mag|phase|ones] chunk stride
N_QBA = 1                # query blocks using the additive (scatter) path
MW = (N_QB - N_QBA) * N_G * GW   # dense multiplier columns per core

f32 = mybir.dt.float32
f16 = mybir.dt.float16
bf16 = mybir.dt.bfloat16
i16 = mybir.dt.int16
AF = mybir.ActivationFunctionType
ALU = mybir.AluOpType

_cache = {}
LAST_RESULTS = None


def _build(slots_a):
    tot_a = N_QBA * N_G * slots_a
    nc = bacc.Bacc("TRN2", target_bir_lowering=False, debug=False,
                   num_devices=CORES)
    xt_d = nc.dram_tensor("xt", (128, N), f16, kind="ExternalInput")
    xtq_d = nc.dram_tensor("xtq", (128, NQ), f16, kind="ExternalInput")
    mp_d = nc.dram_tensor("mp", (128, N_KC * MPW), bf16, kind="ExternalInput")
    mm_d = nc.dram_tensor("mmul", (128, MW), bf16, kind="ExternalInput")
    eidxa_d = nc.dram_tensor("eidxa", (128, tot_a), i16, kind="ExternalInput")
    esba_d = nc.dram_tensor("esba", (128, tot_a), f16, kind="ExternalInput")
    out_d = nc.dram_tensor("out", (NQ, 128), f32, kind="ExternalOutput")

    with tile.TileContext(nc) as tc, \
         tc.tile_pool(name="persist", bufs=1) as pers:
        xt = pers.tile([128, N], f16, tag="xt")
        xtq = pers.tile([128, NQ], f16, tag="xtq")
        mp = pers.tile([128, N_KC * MPW], bf16, tag="mp")
        eidxa = pers.tile([128, tot_a], i16, tag="eidxa")
        esba = pers.tile([128, tot_a], f16, tag="esba")

        # Ramp-critical inputs first; NOTHING on the scalar queue (DGE
        # configs there would serialize the ACT sequencer ahead of the
        # first exp). gpsimd: scatter tables, xt0, mp; sync: xtq + xt
        # tail. M quads stream later on both (interleaved below).
        nc.gpsimd.dma_start(out=eidxa[:], in_=eidxa_d[:])
        nc.gpsimd.dma_start(out=esba[:], in_=esba_d[:])
        nc.sync.dma_start(out=xtq[:], in_=xtq_d[:])
        CW = N // 4
        nc.gpsimd.dma_start(out=xt[:, 0:CW], in_=xt_d[:, 0:CW])
        for h in range(1, 4):
            a, b = h * CW, (h + 1) * CW
            nc.sync.dma_start(out=xt[:, a:b], in_=xt_d[:, a:b])
        MPC = [(0, 16), (16, 40), (40, 64)]
        for a, b in MPC:
            nc.gpsimd.dma_start(out=mp[:, a * MPW:b * MPW],
                                in_=mp_d[:, a * MPW:b * MPW])

        with tc.tile_pool(name="qk_ps", bufs=3, space="PSUM") as qkp, \
             tc.tile_pool(name="pv_ps", bufs=1, space="PSUM") as pvp, \
             tc.tile_pool(name="mmul", bufs=6) as mmp, \
             tc.tile_pool(name="bias", bufs=8) as biasp, \
             tc.tile_pool(name="tmp", bufs=4) as tmpp, \
             tc.tile_pool(name="praw", bufs=10) as prawp, \
             tc.tile_pool(name="psb", bufs=6) as psbp, \
             tc.tile_pool(name="epi", bufs=2) as epip:

            # M quad tiles [128, 4096] for qb1..3 stream on the
            # gpsimd/sync DGE queues (alternating) in consumption order;
            # the 6-deep pool self-paces prefetch.
            m_tiles = {}

            def dma_m(qb, q):
                m = mmp.tile([128, QUAD * GW], bf16, tag="m")
                c0 = ((qb - N_QBA) * N_Q4 + q) * QUAD * GW
                i = (qb - N_QBA) * N_Q4 + q
                eng = nc.gpsimd if i % 2 == 0 else nc.sync
                eng.dma_start(out=m[:], in_=mm_d[:, c0:c0 + QUAD * GW])
                m_tiles[(qb, q)] = m

            for qb in range(N_QBA, N_QB):
                for q in range(N_Q4):
                    dma_m(qb, q)

            # Scatter the qb0 bias tiles up front — they depend only on
            # the (tiny, early) index tables, so GpSimd fills the 8-deep
            # bias pool before the QK results even arrive.
            bias_tiles = {}

            def emit_scatter(g):
                bias_t = biasp.tile([128, GW], f16, tag="bias_t")
                off = g * slots_a
                nc.gpsimd.local_scatter(
                    bias_t[:], esba[:, off:off + slots_a],
                    eidxa[:, off:off + slots_a],
                    channels=128, num_elems=GW, num_idxs=slots_a)
                bias_tiles[g] = bias_t

            for g in range(2 * QUAD):
                emit_scatter(g)

            def emit_qk(qb, q):
                """QK matmuls for one quad; returns the 4 psum tiles."""
                tiles = []
                for gl in range(QUAD):
                    g = q * QUAD + gl
                    qk = qkp.tile([128, GW], f32, tag="qk")
                    for j in range(KCG):
                        kc = g * KCG + j
                        nc.tensor.matmul(
                            out=qk[:, j * QB_W:(j + 1) * QB_W],
                            lhsT=xt[:, kc * 128:(kc + 1) * 128],
                            rhs=xtq[:, qb * QB_W:(qb + 1) * QB_W],
                            start=True, stop=True)
                    tiles.append(qk)
                return tiles

            pend = None
            for qb in range(N_QB):
                pv0 = pvp.tile([128, 129], f32, tag="pv0")
                pv1 = pvp.tile([128, 129], f32, tag="pv1")
                for q in range(N_Q4):
                    qk_tiles = pend if pend is not None else emit_qk(qb, q)
                    pend = None
                    psb_tiles = []
                    if qb < N_QBA:
                        for gl in range(QUAD):
                            g = q * QUAD + gl
                            tmp = tmpp.tile([128, GW], f32, tag="tmp")
                            nc.vector.tensor_tensor(
                                out=tmp[:], in0=qk_tiles[gl][:],
                                in1=bias_tiles.pop(g)[:], op=ALU.add)
                            gn = g + 2 * QUAD
                            if gn < N_G:
                                emit_scatter(gn)
                            p_sb = psbp.tile([128, GW], bf16, tag="p_sb")
                            nc.scalar.activation(out=p_sb[:], in_=tmp[:],
                                                 func=AF.Exp)
                            psb_tiles.append(p_sb)
                    else:
                        m_q = m_tiles[(qb, q)]
                        for gl in range(QUAD):
                            p_raw = prawp.tile([128, GW], bf16, tag="p_raw")
                            nc.scalar.activation(out=p_raw[:],
                                                 in_=qk_tiles[gl][:],
                                                 func=AF.Exp)
                            p_sb = psbp.tile([128, GW], bf16, tag="p_sb")
                            nc.vector.tensor_tensor(
                                out=p_sb[:], in0=p_raw[:],
                                in1=m_q[:, gl * GW:(gl + 1) * GW],
                                op=ALU.mult)
                            psb_tiles.append(p_sb)
                    # queue next quad's QK ahead of this quad's PV so the
                    # tensor engine never waits on the exp
                    if q + 1 < N_Q4:
                        pend = emit_qk(qb, q + 1)
                    elif qb + 1 < N_QB:
                        pend = emit_qk(qb + 1, 0)
                    for gl in range(QUAD):
                        g = q * QUAD + gl
                        for j in range(KCG):
                            kc = g * KCG + j
                            col = j * QB_W
                            for qs, pv in ((0, pv0), (1, pv1)):
                                nc.tensor.matmul(
                                    out=pv[:],
                                    lhsT=psb_tiles[gl][:, col + qs * 128:
                                                       col + (qs + 1) * 128],
                                    rhs=mp[:, kc * MPW:kc * MPW + 2 * D + 1],
                                    start=(kc == 0), stop=(kc == N_KC - 1))
                for qs, pv in ((0, pv0), (1, pv1)):
                    rec = epip.tile([128, 1], f32, tag=f"rec{qs}")
                    nc.vector.reciprocal(out=rec[:], in_=pv[:, 128:129])
                    o_t = epip.tile([128, 128], f32, tag=f"o_t{qs}")
                    nc.vector.tensor_scalar(o_t[:], pv[:, 0:128], rec[:], None,
                                            ALU.mult)
                    r0 = qb * QB_W + qs * 128
                    nc.sync.dma_start(out=out_d[r0:r0 + 128, :], in_=o_t[:])

    nc.compile()
    return nc


def _prep_edges(src, dst, vals):
    """Bucket pre-deduped additive-class (qb0) edges into scatter layout.

    cell = (core, g, p): g = dst group, p = dst % 128; scattered column
    inside the [128, 1024] group tile is ((dst % 512) // 128) * 256 +
    src % 256."""
    core = src // NQ
    g = dst // (KCG * KC)
    p = dst % 128
    col = ((dst % (KCG * KC)) // KC) * QB_W + (src % QB_W)

    cell = (core * N_G + g) * 128 + p
    o2 = np.argsort(cell, kind="stable")
    cell_s = cell[o2]
    first = np.r_[True, cell_s[1:] != cell_s[:-1]]
    run_id = np.cumsum(first) - 1
    run_start = np.flatnonzero(first)
    slot = np.arange(len(cell_s)) - run_start[run_id]
    slots = int(max(int(slot.max()) + 1 if len(slot) else 1, 4))
    slots = (slots + 1) // 2 * 2  # even

    tot = N_G * slots
    eidx = np.full((CORES, 128, tot), -1, dtype=np.int16)
    esb = np.zeros((CORES, 128, tot), dtype=np.float16)
    cs, gs, ps = core[o2], g[o2], p[o2]
    off = gs * slots + slot
    eidx[cs, ps, off] = col[o2].astype(np.int16)
    esb[cs, ps, off] = vals[o2].astype(np.float16)
    return eidx, esb, slots


def kernel(mag, phase, edge_index, edge_attr, W, b):
    global LAST_RESULTS
    mag = np.asarray(mag, dtype=np.float32)
    phase = np.asarray(phase, dtype=np.float32)
    W = np.asarray(W, dtype=np.float32)
    bv = np.asarray(b, dtype=np.float32)

    # trig features, packed transposed: xt[[cos|sin] x d, node]
    c = (mag * np.cos(phase)).astype(np.float16)
    s = (mag * np.sin(phase)).astype(np.float16)
    xt = np.ascontiguousarray(np.concatenate([c.T, s.T], axis=0))  # [128, N]

    # PV value matrix per key chunk: [mag | phase | 1 | pad] stride 132
    mp = np.zeros((128, N_KC, MPW), dtype=np.float32)
    mp[:, :, 0:D] = mag.reshape(N_KC, 128, D).transpose(1, 0, 2)
    mp[:, :, D:2 * D] = phase.reshape(N_KC, 128, D).transpose(1, 0, 2)
    mp[:, :, 2 * D] = 1.0
    mp = mp.reshape(128, N_KC * MPW).astype(ml_dtypes.bfloat16)

    # scalar edge scores: sum_h (edge_attr @ W.T + b)[:, h]; dedup last-wins
    es_all = (np.asarray(edge_attr, dtype=np.float64) @
              W.astype(np.float64).sum(axis=0) + bv.astype(np.float64).sum())
    src = np.asarray(edge_index[0], dtype=np.int64)
    dst = np.asarray(edge_index[1], dtype=np.int64)
    keys = src * N + dst
    order = np.argsort(keys, kind="stable")
    ks = keys[order]
    run_last = np.flatnonzero(np.r_[ks[1:] != ks[:-1], True])
    kept = order[run_last]  # stable sort => last occurrence per duplicate key
    src, dst, es = src[kept], dst[kept], es_all[kept]

    # class split by src query block: qb0 additive (device scatter),
    # qb1..3 multiplicative (dense M = exp(bias) from host)
    qbi = (src % NQ) // QB_W
    is_a = qbi < N_QBA

    eidxa, esba, slots_a = _prep_edges(src[is_a], dst[is_a], es[is_a])

    sm, dm, em = src[~is_a], dst[~is_a], np.exp(es[~is_a])
    mmul = np.full((CORES, 128, MW), 0x3F80, dtype=np.uint16)  # bf16 1.0
    mmul = mmul.view(ml_dtypes.bfloat16)
    col = ((dm // (KCG * KC)) * GW + ((dm % (KCG * KC)) // KC) * QB_W +
           (sm % QB_W))
    qbm = (sm % NQ) // QB_W - N_QBA
    mmul[sm // NQ, dm % 128, qbm * (N_G * GW) + col] = \
        em.astype(ml_dtypes.bfloat16)

    if slots_a not in _cache:
        _cache[slots_a] = _build(slots_a)
    nc = _cache[slots_a]

    in_maps = []
    for cid in range(CORES):
        in_maps.append({
            "xt": xt,
            "xtq": np.ascontiguousarray(xt[:, cid * NQ:(cid + 1) * NQ]),
            "mp": mp,
            "mmul": mmul[cid],
            "eidxa": np.ascontiguousarray(eidxa[cid]),
            "esba": np.ascontiguousarray(esba[cid]),
        })
    res = run_bass_kernel_spmd(nc, in_maps, core_ids=list(range(CORES)))
    LAST_RESULTS = res

    new_mag = np.empty((N, D), dtype=np.float32)
    new_phase = np.empty((N, D), dtype=np.float32)
    for cid in range(CORES):
        o = res.results[cid]["out"]
        new_mag[cid * NQ:(cid + 1) * NQ] = o[:, 0:D]
        new_phase[cid * NQ:(cid + 1) * NQ] = o[:, D:2 * D]
    return new_mag, new_phase


# revision 6
# speedup vs baseline: 1.0373x; 1.0373x over previous
"""ComplexPolarAttention Trainium2 kernel (8-core SPMD, row-sharded).

Math (matching the reference):
  c = mag*cos(phase); s = mag*sin(phase)
  scores = c@c.T + s@s.T + bias     (bias: sparse edge scatter, last-dup-wins)
  attn = softmax(scores, axis=1)
  out = (attn@mag, attn@phase)

Design (v4, all-multiplicative bias):
  exp(S+B) = exp(S) * M with M = exp(B) provided DENSE from the host in
  bf16 (1.0 everywhere except the ~0.4% edge cells).  Host precomputes
  trig features packed transposed as xt [128 feat, 8192 nodes] f16, the
  PV value matrix mp [128, 64*132] bf16 ([mag|phase|ones] per key
  chunk), and M per core [128, 65536] bf16.

  The scalar (ACT) queue carries NO input DMA configs; ACT does nothing
  but stream 64 back-to-back [128,1024] exps (~1.0us each) — it is the
  pacing engine.  GpSimd runs NO compute (concurrent GpSimd activity
  was measured to clock-throttle every engine ~20%); its SWDGE queue
  only streams M/mp.  QK matmuls for quad q+1 are emitted before PV
  matmuls of quad q so the tensor engine never stalls on the exp.  The
  deep p_raw pool lets ACT run up to 16 exps ahead of the DVE mults,
  riding out the early-window DMA crunch on M.

Device per core (1024 query rows), per group g (= 4 key chunks x 256
queries = [128 dst, 1024] score tile):
  PE:   S^T group tile = xt_kc.T @ xtq   (f16, PSUM f32)
  ACT:  p_raw = exp(S^T) straight from PSUM -> bf16 SBUF
  DVE:  p = p_raw * M_g   (bf16 2x mode)
  PE:   pv[128 q, 129] += p_chunk.T @ [mag|phase|ones]  (col 128 =
        softmax denominator); epilogue divides and DMAs out.
"""
import os
import sys

sys.path.insert(0, "/opt/trn_rl_repo")

# The NTFF profile hook module is missing from this image's antenv package;
# bass_utils imports it unconditionally when tracing. Create it if absent so
# BASS_TRACE=1 works (degrades silently if dirs are read-only).
_HOOK_SRC = '''_hook = None

def set_axon_ntff_profile_hook(hook):
    global _hook
    _hook = hook

def get_axon_ntff_profile_hook():
    return _hook
'''
for _d in ("/opt/trn_rl_repo/antenv", "/root/.axon_site/_ro/trn_rl_repo/antenv"):
    try:
        _p = os.path.join(_d, "axon_hooks.py")
        if os.path.isdir(_d) and not os.path.exists(_p):
            with open(_p, "w") as _f:
                _f.write(_HOOK_SRC)
    except OSError:
        pass

import numpy as np
import ml_dtypes

import concourse.bass as bass
import concourse.mybir as mybir
import concourse.tile as tile
from concourse import bacc
from concourse.bass_utils import run_bass_kernel_spmd

N, D, E, EDGE_DIM = 8192, 64, 262144, 4
CORES = 8
NQ = N // CORES          # 1024 query rows per core
QB_W = 256               # query block width
N_QB = NQ // QB_W        # 4 query blocks per core
KC = 128                 # key chunk (dst) width
N_KC = N // KC           # 64 key chunks
KCG = 4                  # key chunks per group
N_G = N_KC // KCG        # 16 groups per qb
GW = KCG * QB_W          # 1024 = group tile width
QUAD = 4                 # groups per QK-emission batch
N_Q4 = N_G // QUAD       # 4 quads per qb
MPW = 132                # padded [mag|phase|ones] chunk stride
MW = N_QB * N_G * GW     # dense multiplier columns per core (65536)

f32 = mybir.dt.float32
f16 = mybir.dt.float16
bf16 = mybir.dt.bfloat16
AF = mybir.ActivationFunctionType
ALU = mybir.AluOpType

_cache = {}
LAST_RESULTS = None


def _build():
    nc = bacc.Bacc("TRN2", target_bir_lowering=False, debug=False,
                   num_devices=CORES)
    xt_d = nc.dram_tensor("xt", (128, N), f16, kind="ExternalInput")
    xtq_d = nc.dram_tensor("xtq", (128, NQ), f16, kind="ExternalInput")
    mp_d = nc.dram_tensor("mp", (128, N_KC * MPW), bf16, kind="ExternalInput")
    mm_d = nc.dram_tensor("mmul", (128, MW), bf16, kind="ExternalInput")
    out_d = nc.dram_tensor("out", (NQ, 128), f32, kind="ExternalOutput")

    with tile.TileContext(nc) as tc, \
         tc.tile_pool(name="persist", bufs=1) as pers:
        xt = pers.tile([128, N], f16, tag="xt")
        xtq = pers.tile([128, NQ], f16, tag="xtq")
        mp = pers.tile([128, N_KC * MPW], bf16, tag="mp")

        # Ramp-critical first: xt0 on gpsimd || xtq on sync so the first
        # QK can fire ~14us in; then xt tail on sync while gpsimd takes
        # the first M quads + early mp chunks.  Nothing on scalar.
        nc.sync.dma_start(out=xtq[:], in_=xtq_d[:])
        CW = N // 4
        nc.gpsimd.dma_start(out=xt[:, 0:CW], in_=xt_d[:, 0:CW])
        for h in range(1, 4):
            a, b = h * CW, (h + 1) * CW
            nc.sync.dma_start(out=xt[:, a:b], in_=xt_d[:, a:b])

        with tc.tile_pool(name="qk_ps", bufs=3, space="PSUM") as qkp, \
             tc.tile_pool(name="pv_ps", bufs=1, space="PSUM") as pvp, \
             tc.tile_pool(name="mmul", bufs=6) as mmp, \
             tc.tile_pool(name="praw", bufs=16) as prawp, \
             tc.tile_pool(name="psb", bufs=8) as psbp, \
             tc.tile_pool(name="epi", bufs=2) as epip:

            m_tiles = {}

            def dma_m(qb, q, eng):
                m = mmp.tile([128, QUAD * GW], bf16, tag="m")
                c0 = (qb * N_Q4 + q) * QUAD * GW
                eng.dma_start(out=m[:], in_=mm_d[:, c0:c0 + QUAD * GW])
                m_tiles[(qb, q)] = m

            def dma_mp(k0, k1, eng):
                eng.dma_start(out=mp[:, k0 * MPW:k1 * MPW],
                              in_=mp_d[:, k0 * MPW:k1 * MPW])

            # Interleave M quads and mp chunks across both queues in
            # consumption order.  gpsimd (147GB/s): M(0,0), mp[kc0..7],
            # M(0,1), mp[kc8..31], M(0,2), mp[kc32..63], then odd quads.
            # sync (123GB/s, after xtq+xt tail): M(0,3) and even quads.
            dma_m(0, 0, nc.gpsimd)
            dma_mp(0, 8, nc.gpsimd)
            dma_m(0, 1, nc.gpsimd)
            dma_mp(8, 32, nc.gpsimd)
            dma_m(0, 2, nc.gpsimd)
            dma_mp(32, 64, nc.gpsimd)
            dma_m(0, 3, nc.sync)
            rest = [(qb, q) for qb in range(1, N_QB) for q in range(N_Q4)]
            for i, (qb, q) in enumerate(rest):
                dma_m(qb, q, nc.gpsimd if i % 2 == 0 else nc.sync)

            def emit_qk(qb, q):
                """QK matmuls for one quad; returns the 4 psum tiles."""
                tiles = []
                for gl in range(QUAD):
                    g = q * QUAD + gl
                    qk = qkp.tile([128, GW], f32, tag="qk")
                    for j in range(KCG):
                        kc = g * KCG + j
                        nc.tensor.matmul(
                            out=qk[:, j * QB_W:(j + 1) * QB_W],
                            lhsT=xt[:, kc * 128:(kc + 1) * 128],
                            rhs=xtq[:, qb * QB_W:(qb + 1) * QB_W],
                            start=True, stop=True)
                    tiles.append(qk)
                return tiles

            pend = None
            for qb in range(N_QB):
                pv0 = pvp.tile([128, 129], f32, tag="pv0")
                pv1 = pvp.tile([128, 129], f32, tag="pv1")
                for q in range(N_Q4):
                    qk_tiles = pend if pend is not None else emit_qk(qb, q)
                    pend = None
                    m_q = m_tiles[(qb, q)]
                    psb_tiles = []
                    for gl in range(QUAD):
                        p_raw = prawp.tile([128, GW], bf16, tag="p_raw")
                        nc.scalar.activation(out=p_raw[:],
                                             in_=qk_tiles[gl][:],
                                             func=AF.Exp)
                        p_sb = psbp.tile([128, GW], bf16, tag="p_sb")
                        nc.vector.tensor_tensor(
                            out=p_sb[:], in0=p_raw[:],
                            in1=m_q[:, gl * GW:(gl + 1) * GW],
                            op=ALU.mult)
                        psb_tiles.append(p_sb)
                    # queue next quad's QK ahead of this quad's PV so the
                    # tensor engine never waits on the exp
                    if q + 1 < N_Q4:
                        pend = emit_qk(qb, q + 1)
                    elif qb + 1 < N_QB:
                        pend = emit_qk(qb + 1, 0)
                    for gl in range(QUAD):
                        g = q * QUAD + gl
                        for j in range(KCG):
                            kc = g * KCG + j
                            col = j * QB_W
                            for qs, pv in ((0, pv0), (1, pv1)):
                                nc.tensor.matmul(
                                    out=pv[:],
                                    lhsT=psb_tiles[gl][:, col + qs * 128:
                                                       col + (qs + 1) * 128],
                                    rhs=mp[:, kc * MPW:kc * MPW + 2 * D + 1],
                                    start=(kc == 0), stop=(kc == N_KC - 1))
                for qs, pv in ((0, pv0), (1, pv1)):
                    rec = epip.tile([128, 1], f32, tag=f"rec{qs}")
                    nc.vector.reciprocal(out=rec[:], in_=pv[:, 128:129])
                    o_t = epip.tile([128, 128], f32, tag=f"o_t{qs}")
                    nc.vector.tensor_scalar(o_t[:], pv[:, 0:128], rec[:], None,
                                            ALU.mult)
                    r0 = qb * QB_W + qs * 128
                    nc.sync.dma_start(out=out_d[r0:r0 + 128, :], in_=o_t[:])

    nc.compile()
    return nc


def kernel(mag, phase, edge_index, edge_attr, W, b):
    global LAST_RESULTS
    mag = np.asarray(mag, dtype=np.float32)
    phase = np.asarray(phase, dtype=np.float32)
    W = np.asarray(W, dtype=np.float32)
    bv = np.asarray(b, dtype=np.float32)

    # trig features, packed transposed: xt[[cos|sin] x d, node]
    c = (mag * np.cos(phase)).astype(np.float16)
    s = (mag * np.sin(phase)).astype(np.float16)
    xt = np.ascontiguousarray(np.concatenate([c.T, s.T], axis=0))  # [128, N]

    # PV value matrix per key chunk: [mag | phase | 1 | pad] stride 132
    mp = np.zeros((128, N_KC, MPW), dtype=np.float32)
    mp[:, :, 0:D] = mag.reshape(N_KC, 128, D).transpose(1, 0, 2)
    mp[:, :, D:2 * D] = phase.reshape(N_KC, 128, D).transpose(1, 0, 2)
    mp[:, :, 2 * D] = 1.0
    mp = mp.reshape(128, N_KC * MPW).astype(ml_dtypes.bfloat16)

    # scalar edge scores: sum_h (edge_attr @ W.T + b)[:, h]; dedup last-wins
    es_all = (np.asarray(edge_attr, dtype=np.float64) @
              W.astype(np.float64).sum(axis=0) + bv.astype(np.float64).sum())
    src = np.asarray(edge_index[0], dtype=np.int64)
    dst = np.asarray(edge_index[1], dtype=np.int64)
    keys = src * N + dst
    order = np.argsort(keys, kind="stable")
    ks = keys[order]
    run_last = np.flatnonzero(np.r_[ks[1:] != ks[:-1], True])
    kept = order[run_last]  # stable sort => last occurrence per duplicate key
    src, dst, es = src[kept], dst[kept], es_all[kept]

    # dense multiplier M = exp(bias): [core][128 p, (qb, g, j, srccol) cols]
    em = np.exp(es)
    mmul = np.full((CORES, 128, MW), 0x3F80, dtype=np.uint16)  # bf16 1.0
    mmul = mmul.view(ml_dtypes.bfloat16)
    col = ((dst // (KCG * KC)) * GW + ((dst % (KCG * KC)) // KC) * QB_W +
           (src % QB_W))
    qbi = (src % NQ) // QB_W
    mmul[src // NQ, dst % 128, qbi * (N_G * GW) + col] = \
        em.astype(ml_dtypes.bfloat16)

    if "nc" not in _cache:
        _cache["nc"] = _build()
    nc = _cache["nc"]

    in_maps = []
    for cid in range(CORES):
        in_maps.append({
            "xt": xt,
            "xtq": np.ascontiguousarray(xt[:, cid * NQ:(cid + 1) * NQ]),
            "mp": mp,
            "mmul": mmul[cid],
        })
    res = run_bass_kernel_spmd(nc, in_maps, core_ids=list(range(CORES)))
    LAST_RESULTS = res

    new_mag = np.empty((N, D), dtype=np.float32)
    new_phase = np.empty((N, D), dtype=np.float32)
    for cid in range(CORES):
        o = res.results[cid]["out"]
        new_mag[cid * NQ:(cid + 1) * NQ] = o[:, 0:D]
        new_phase[cid * NQ:(cid + 1) * NQ] = o[:, D:2 * D]
    return new_mag, new_phase


# revision 7
# speedup vs baseline: 1.0824x; 1.0435x over previous
"""ComplexPolarAttention Trainium2 kernel (8-core SPMD, row-sharded).

Math (matching the reference):
  c = mag*cos(phase); s = mag*sin(phase)
  scores = c@c.T + s@s.T + bias     (bias: sparse edge scatter, last-dup-wins)
  attn = softmax(scores, axis=1)
  out = (attn@mag, attn@phase)

Design (v4, all-multiplicative bias):
  exp(S+B) = exp(S) * M with M = exp(B) provided DENSE from the host in
  bf16 (1.0 everywhere except the ~0.4% edge cells).  Host precomputes
  trig features packed transposed as xt [128 feat, 8192 nodes] f16, the
  PV value matrix mp [128, 64*132] bf16 ([mag|phase|ones] per key
  chunk), and M per core [128, 65536] bf16.

  The scalar (ACT) queue carries NO input DMA configs; ACT does nothing
  but stream 64 back-to-back [128,1024] exps (~1.0us each) — it is the
  pacing engine.  GpSimd runs NO compute (concurrent GpSimd activity
  was measured to clock-throttle every engine ~20%); its SWDGE queue
  only streams M/mp.  QK matmuls for quad q+1 are emitted before PV
  matmuls of quad q so the tensor engine never stalls on the exp.  The
  deep p_raw pool lets ACT run up to 16 exps ahead of the DVE mults,
  riding out the early-window DMA crunch on M.

Device per core (1024 query rows), per group g (= 4 key chunks x 256
queries = [128 dst, 1024] score tile):
  PE:   S^T group tile = xt_kc.T @ xtq   (f16, PSUM f32)
  ACT:  p_raw = exp(S^T) straight from PSUM -> bf16 SBUF
  DVE:  p = p_raw * M_g   (bf16 2x mode)
  PE:   pv[128 q, 129] += p_chunk.T @ [mag|phase|ones]  (col 128 =
        softmax denominator); epilogue divides and DMAs out.
"""
import os
import sys

sys.path.insert(0, "/opt/trn_rl_repo")

# The NTFF profile hook module is missing from this image's antenv package;
# bass_utils imports it unconditionally when tracing. Create it if absent so
# BASS_TRACE=1 works (degrades silently if dirs are read-only).
_HOOK_SRC = '''_hook = None

def set_axon_ntff_profile_hook(hook):
    global _hook
    _hook = hook

def get_axon_ntff_profile_hook():
    return _hook
'''
for _d in ("/opt/trn_rl_repo/antenv", "/root/.axon_site/_ro/trn_rl_repo/antenv"):
    try:
        _p = os.path.join(_d, "axon_hooks.py")
        if os.path.isdir(_d) and not os.path.exists(_p):
            with open(_p, "w") as _f:
                _f.write(_HOOK_SRC)
    except OSError:
        pass

import numpy as np
import ml_dtypes

import concourse.bass as bass
import concourse.mybir as mybir
import concourse.tile as tile
from concourse import bacc
from concourse.bass_utils import run_bass_kernel_spmd

N, D, E, EDGE_DIM = 8192, 64, 262144, 4
CORES = 8
NQ = N // CORES          # 1024 query rows per core
QB_W = 256               # query block width
N_QB = NQ // QB_W        # 4 query blocks per core
KC = 128                 # key chunk (dst) width
N_KC = N // KC           # 64 key chunks
KCG = 4                  # key chunks per group
N_G = N_KC // KCG        # 16 groups per qb
GW = KCG * QB_W          # 1024 = group tile width
QUAD = 4                 # groups per QK-emission batch
N_Q4 = N_G // QUAD       # 4 quads per qb
MPW = 132                # padded [mag|phase|ones] chunk stride
MW = N_QB * N_G * GW     # dense multiplier columns per core (65536)

f32 = mybir.dt.float32
f16 = mybir.dt.float16
bf16 = mybir.dt.bfloat16
AF = mybir.ActivationFunctionType
ALU = mybir.AluOpType

_cache = {}
LAST_RESULTS = None


def _build():
    nc = bacc.Bacc("TRN2", target_bir_lowering=False, debug=False,
                   num_devices=CORES)
    xt_d = nc.dram_tensor("xt", (128, N), f16, kind="ExternalInput")
    xtq_d = nc.dram_tensor("xtq", (128, NQ), f16, kind="ExternalInput")
    mp_d = nc.dram_tensor("mp", (128, N_KC * MPW), bf16, kind="ExternalInput")
    mm_d = nc.dram_tensor("mmul", (128, MW), bf16, kind="ExternalInput")
    out_d = nc.dram_tensor("out", (NQ, 128), f32, kind="ExternalOutput")

    with tile.TileContext(nc) as tc, \
         tc.tile_pool(name="persist", bufs=1) as pers:
        xt = pers.tile([128, N], f16, tag="xt")
        xtq = pers.tile([128, NQ], f16, tag="xtq")
        mp = pers.tile([128, N_KC * MPW], bf16, tag="mp")

        # Ramp-critical first: xt0 on gpsimd || xtq on sync so the first
        # QK can fire ~14us in; then xt tail on sync while gpsimd takes
        # the first M quads + early mp chunks.  Nothing on scalar.
        nc.sync.dma_start(out=xtq[:], in_=xtq_d[:])
        CW = N // 4
        nc.gpsimd.dma_start(out=xt[:, 0:CW], in_=xt_d[:, 0:CW])
        for h in range(1, 4):
            a, b = h * CW, (h + 1) * CW
            nc.sync.dma_start(out=xt[:, a:b], in_=xt_d[:, a:b])

        with tc.tile_pool(name="qk_ps", bufs=3, space="PSUM") as qkp, \
             tc.tile_pool(name="pv_ps", bufs=1, space="PSUM") as pvp, \
             tc.tile_pool(name="mmul", bufs=6) as mmp, \
             tc.tile_pool(name="praw", bufs=16) as prawp, \
             tc.tile_pool(name="psb", bufs=8) as psbp, \
             tc.tile_pool(name="epi", bufs=2) as epip:

            m_tiles = {}

            def dma_m(qb, q, eng):
                m = mmp.tile([128, QUAD * GW], bf16, tag="m")
                c0 = (qb * N_Q4 + q) * QUAD * GW
                eng.dma_start(out=m[:], in_=mm_d[:, c0:c0 + QUAD * GW])
                m_tiles[(qb, q)] = m

            def dma_mp(k0, k1, eng):
                eng.dma_start(out=mp[:, k0 * MPW:k1 * MPW],
                              in_=mp_d[:, k0 * MPW:k1 * MPW])

            # Interleave M quads and mp chunks across both queues in
            # consumption order.  gpsimd (147GB/s): M(0,0), mp[kc0..7],
            # M(0,1), mp[kc8..31], M(0,2), mp[kc32..63], then odd quads.
            # sync (123GB/s, after xtq+xt tail): M(0,3) and even quads.
            dma_m(0, 0, nc.gpsimd)
            dma_mp(0, 8, nc.gpsimd)
            dma_m(0, 1, nc.gpsimd)
            dma_mp(8, 32, nc.gpsimd)
            dma_m(0, 2, nc.gpsimd)
            dma_mp(32, 64, nc.gpsimd)
            dma_m(0, 3, nc.sync)
            rest = [(qb, q) for qb in range(1, N_QB) for q in range(N_Q4)]
            for i, (qb, q) in enumerate(rest):
                dma_m(qb, q, nc.gpsimd if i % 2 == 0 else nc.sync)

            def emit_qk(qb, q):
                """QK matmuls for one quad; returns the 4 psum tiles."""
                tiles = []
                for gl in range(QUAD):
                    g = q * QUAD + gl
                    qk = qkp.tile([128, GW], f32, tag="qk")
                    for j in range(KCG):
                        kc = g * KCG + j
                        nc.tensor.matmul(
                            out=qk[:, j * QB_W:(j + 1) * QB_W],
                            lhsT=xt[:, kc * 128:(kc + 1) * 128],
                            rhs=xtq[:, qb * QB_W:(qb + 1) * QB_W],
                            start=True, stop=True)
                    tiles.append(qk)
                return tiles

            pend = None
            for qb in range(N_QB):
                pv0 = pvp.tile([128, 129], f32, tag="pv0")
                pv1 = pvp.tile([128, 129], f32, tag="pv1")
                for q in range(N_Q4):
                    qk_tiles = pend if pend is not None else emit_qk(qb, q)
                    pend = None
                    m_q = m_tiles[(qb, q)]
                    psb_tiles = []
                    for gl in range(QUAD):
                        g_all = (qb * N_G + q * QUAD + gl)
                        p_raw = prawp.tile([128, GW], bf16, tag="p_raw")
                        nc.scalar.activation(out=p_raw[:],
                                             in_=qk_tiles[gl][:],
                                             func=AF.Exp)
                        p_sb = psbp.tile([128, GW], bf16, tag="p_sb")
                        # Scheduler hint: this mult cannot really start
                        # before its M quad lands (~16us ramp + ~1.05us
                        # per group of ACT pacing).  Without it the
                        # scheduler assumes M arrives instantly and
                        # interleaves PV ahead of the next QK quad in
                        # the PE stream, head-of-line blocking QK when M
                        # is late and starving ACT.
                        with tc.tile_wait_until((15.5 + 1.05 * g_all) / 1000):
                            nc.vector.tensor_tensor(
                                out=p_sb[:], in0=p_raw[:],
                                in1=m_q[:, gl * GW:(gl + 1) * GW],
                                op=ALU.mult)
                        psb_tiles.append(p_sb)
                    # queue next quad's QK ahead of this quad's PV so the
                    # tensor engine never waits on the exp
                    if q + 1 < N_Q4:
                        pend = emit_qk(qb, q + 1)
                    elif qb + 1 < N_QB:
                        pend = emit_qk(qb + 1, 0)
                    for gl in range(QUAD):
                        g = q * QUAD + gl
                        for j in range(KCG):
                            kc = g * KCG + j
                            col = j * QB_W
                            for qs, pv in ((0, pv0), (1, pv1)):
                                nc.tensor.matmul(
                                    out=pv[:],
                                    lhsT=psb_tiles[gl][:, col + qs * 128:
                                                       col + (qs + 1) * 128],
                                    rhs=mp[:, kc * MPW:kc * MPW + 2 * D + 1],
                                    start=(kc == 0), stop=(kc == N_KC - 1))
                for qs, pv in ((0, pv0), (1, pv1)):
                    rec = epip.tile([128, 1], f32, tag=f"rec{qs}")
                    nc.vector.reciprocal(out=rec[:], in_=pv[:, 128:129])
                    o_t = epip.tile([128, 128], f32, tag=f"o_t{qs}")
                    nc.vector.tensor_scalar(o_t[:], pv[:, 0:128], rec[:], None,
                                            ALU.mult)
                    r0 = qb * QB_W + qs * 128
                    nc.sync.dma_start(out=out_d[r0:r0 + 128, :], in_=o_t[:])

    nc.compile()
    return nc


def kernel(mag, phase, edge_index, edge_attr, W, b):
    global LAST_RESULTS
    mag = np.asarray(mag, dtype=np.float32)
    phase = np.asarray(phase, dtype=np.float32)
    W = np.asarray(W, dtype=np.float32)
    bv = np.asarray(b, dtype=np.float32)

    # trig features, packed transposed: xt[[cos|sin] x d, node]
    c = (mag * np.cos(phase)).astype(np.float16)
    s = (mag * np.sin(phase)).astype(np.float16)
    xt = np.ascontiguousarray(np.concatenate([c.T, s.T], axis=0))  # [128, N]

    # PV value matrix per key chunk: [mag | phase | 1 | pad] stride 132
    mp = np.zeros((128, N_KC, MPW), dtype=np.float32)
    mp[:, :, 0:D] = mag.reshape(N_KC, 128, D).transpose(1, 0, 2)
    mp[:, :, D:2 * D] = phase.reshape(N_KC, 128, D).transpose(1, 0, 2)
    mp[:, :, 2 * D] = 1.0
    mp = mp.reshape(128, N_KC * MPW).astype(ml_dtypes.bfloat16)

    # scalar edge scores: sum_h (edge_attr @ W.T + b)[:, h]; dedup last-wins
    es_all = (np.asarray(edge_attr, dtype=np.float64) @
              W.astype(np.float64).sum(axis=0) + bv.astype(np.float64).sum())
    src = np.asarray(edge_index[0], dtype=np.int64)
    dst = np.asarray(edge_index[1], dtype=np.int64)
    keys = src * N + dst
    order = np.argsort(keys, kind="stable")
    ks = keys[order]
    run_last = np.flatnonzero(np.r_[ks[1:] != ks[:-1], True])
    kept = order[run_last]  # stable sort => last occurrence per duplicate key
    src, dst, es = src[kept], dst[kept], es_all[kept]

    # dense multiplier M = exp(bias): [core][128 p, (qb, g, j, srccol) cols]
    em = np.exp(es)
    mmul = np.full((CORES, 128, MW), 0x3F80, dtype=np.uint16)  # bf16 1.0
    mmul = mmul.view(ml_dtypes.bfloat16)
    col = ((dst // (KCG * KC)) * GW + ((dst % (KCG * KC)) // KC) * QB_W +
           (src % QB_W))
    qbi = (src % NQ) // QB_W
    mmul[src // NQ, dst % 128, qbi * (N_G * GW) + col] = \
        em.astype(ml_dtypes.bfloat16)

    if "nc" not in _cache:
        _cache["nc"] = _build()
    nc = _cache["nc"]

    in_maps = []
    for cid in range(CORES):
        in_maps.append({
            "xt": xt,
            "xtq": np.ascontiguousarray(xt[:, cid * NQ:(cid + 1) * NQ]),
            "mp": mp,
            "mmul": mmul[cid],
        })
    res = run_bass_kernel_spmd(nc, in_maps, core_ids=list(range(CORES)))
    LAST_RESULTS = res

    new_mag = np.empty((N, D), dtype=np.float32)
    new_phase = np.empty((N, D), dtype=np.float32)
    for cid in range(CORES):
        o = res.results[cid]["out"]
        new_mag[cid * NQ:(cid + 1) * NQ] = o[:, 0:D]
        new_phase[cid * NQ:(cid + 1) * NQ] = o[:, D:2 * D]
    return new_mag, new_phase
